# revision 2
# baseline (speedup 1.0000x reference)
"""MoE block (B=2,S=2048,D=2048,FF=8192,E=16,K=2,C=640) on 8 trn2 cores.

Expert parallelism: 2 experts per core. Gate is data-parallel: each core
computes top-2 + gates for its 512-token stripe in fp32, a small AllGather
replicates [T, 4] = (i0, i1, g0, g1), then every core batch-computes the
capacity positions and builds its 2 experts' slot table with one swizzled
dma_scatter_add. Expert token rows are fetched transposed straight into
SBUF via dma_gather(transpose=True); the FFN runs in bf16 (fp32 accum) and
scatter-adds gate-weighted rows into a per-core partial output that the
host sums (expert-parallel combine/unshard).
"""
import sys
sys.path.insert(0, "/opt/trn_rl_repo")
import numpy as np
import ml_dtypes

import concourse.bass as bass
import concourse.mybir as mybir
import concourse.tile as tile
from concourse import bacc
from concourse.bass_utils import run_bass_kernel_spmd

F32 = mybir.dt.float32
BF16 = mybir.dt.bfloat16
I32 = mybir.dt.int32
I16 = mybir.dt.int16
U32 = mybir.dt.uint32
AL = mybir.AluOpType
ACTF = mybir.ActivationFunctionType

B, S, D, FF, E, K = 2, 2048, 2048, 8192, 16, 2
T = B * S                 # 4096 tokens
C = 640                   # per-expert capacity
NB = T // 128             # 32 token blocks
EL = 2                    # local experts per core
NF = FF // 128            # 64 f-tiles
ND = D // 512             # 4 dd chunks
NCT = C // 128            # 5 capacity tiles per expert
NK = D // 128             # 16 contraction tiles of D
NSL = EL * C              # 1280 local slots
TAB = 1408                # slot table rows (incl dump zone >= NSL)

# consts blob column offsets
CB_TOK = 0
CB_TRI = CB_TOK + NB
CB_ID = CB_TRI + 128
CB_ONE = CB_ID + 128
CB_BASE = CB_ONE + 1
CB_W = CB_BASE + 1

_CACHE = {}


def _build_nc():
    nc = bacc.Bacc(None, target_bir_lowering=False, debug=True)

    xtloc = nc.dram_tensor("xtloc", [D, 512], F32, kind="ExternalInput")
    xb = nc.dram_tensor("xb", [T + 1, D], BF16, kind="ExternalInput")
    wgp = nc.dram_tensor("wgp", [128, NK * E], F32, kind="ExternalInput")
    w1h = nc.dram_tensor("w1h", [EL, NF, 128, NK, 128], BF16, kind="ExternalInput")
    w2h = nc.dram_tensor("w2h", [EL, ND, NF, 128, 512], BF16, kind="ExternalInput")
    b1h = nc.dram_tensor("b1h", [EL, 128, NF], F32, kind="ExternalInput")
    b2d = nc.dram_tensor("b2d", [EL, D], BF16, kind="ExternalInput")
    cb = nc.dram_tensor("cb", [128, CB_W], F32, kind="ExternalInput")

    loc4 = nc.dram_tensor("loc4", [512, 4], F32)
    gath4 = nc.dram_tensor("gath4", [T, 4], F32, addr_space="Shared")
    tab = nc.dram_tensor("tab", [TAB, 64], F32)
    idxd = nc.dram_tensor("idxd", [1, 2 * T], I16)
    tokd = nc.dram_tensor("tokd", [1, NSL], I16)
    exclb = nc.dram_tensor("exclb", [1, 2 * E * NB], F32)

    y = nc.dram_tensor("y", [T, D], F32, kind="ExternalOutput")

    with tile.TileContext(nc) as tc:
        with tc.tile_pool(name="consts", bufs=1) as cp:
            cbs = cp.tile([128, CB_W], F32)
            nc.scalar.dma_start(cbs[:], cb[:])
            tok_sb = cbs[:, CB_TOK:CB_TOK + NB]
            tri_sb = cbs[:, CB_TRI:CB_TRI + 128]
            id_sb = cbs[:, CB_ID:CB_ID + 128]
            ones_sb = cbs[:, CB_ONE:CB_ONE + 1]
            base_sb = cbs[:, CB_BASE:CB_BASE + 1]
            wg_sb = cp.tile([128, NK * E], F32)
            nc.scalar.dma_start(wg_sb[:], wgp[:])
            b1_sb = [cp.tile([128, NF], F32, tag=f"b1_{e}", name=f"b1_{e}")
                     for e in range(EL)]
            b2_sb = [cp.tile([128, D], BF16, tag=f"b2_{e}", name=f"b2_{e}")
                     for e in range(EL)]
            for e in range(EL):
                nc.scalar.dma_start(b1_sb[e][:], b1h[e])
                nc.scalar.dma_start(b2_sb[e][:], b2d[e:e + 1, :].to_broadcast([128, D]))
            gsb = cp.tile([128, EL * NCT], F32)
            toki = cp.tile([128, EL * NCT], I32)

            # ---------------- routing ----------------
            with tc.tile_pool(name="rout", bufs=1) as rp, \
                 tc.tile_pool(name="psr", bufs=1, space="PSUM") as pr:
                # zero-init the slot table (cols 0:2 used)
                zz = rp.tile([128, TAB // 128, 2], F32)
                nc.vector.memset(zz[:], 0.0)
                nc.sync.dma_start(
                    tab[:, 0:2].rearrange("(n p) c -> p n c", p=128), zz[:])

                # gate for the local 512-token stripe (fp32, data-parallel)
                xtg = rp.tile([128, NK, 512], F32)
                nc.sync.dma_start(xtg[:], xtloc[:].rearrange("(k p) c -> p k c",
                                                             p=128))
                locsb = rp.tile([128, 4, E], F32)
                for j in range(4):
                    lg = pr.tile([128, E], F32, tag="lg", bufs=2)
                    for k in range(NK):
                        nc.tensor.matmul(lg[:], lhsT=xtg[:, k, j * 128:(j + 1) * 128],
                                         rhs=wg_sb[:, k * E:(k + 1) * E],
                                         start=(k == 0), stop=(k == NK - 1))
                    nc.vector.tensor_copy(locsb[:, j, :], lg[:])

                # local top-2 + gates
                mx = rp.tile([128, 4, 8], F32)
                mi = rp.tile([128, 4, 8], U32)
                dte = rp.tile([128, 4], F32)
                exd = rp.tile([128, 4], F32)
                den = rp.tile([128, 4], F32)
                g0l = rp.tile([128, 4], F32)
                l4 = rp.tile([128, 4, 4], F32)
                for j in range(4):
                    nc.vector.max(out=mx[:, j, :], in_=locsb[:, j, :])
                    nc.vector.max_index(out=mi[:, j, :], in_max=mx[:, j, :],
                                        in_values=locsb[:, j, :])
                    nc.vector.tensor_tensor(out=dte[:, j:j + 1], in0=mx[:, j, 1:2],
                                            in1=mx[:, j, 0:1], op=AL.subtract)
                nc.scalar.activation(exd[:], dte[:], ACTF.Exp)
                nc.vector.tensor_scalar_add(den[:], exd[:], 1.0)
                nc.vector.reciprocal(g0l[:], den[:])
                nc.vector.tensor_copy(l4[:, :, 0], mi[:, :, 0])
                nc.vector.tensor_copy(l4[:, :, 1], mi[:, :, 1])
                nc.vector.tensor_copy(l4[:, :, 2], g0l[:])
                nc.vector.tensor_tensor(out=l4[:, :, 3], in0=exd[:], in1=g0l[:],
                                        op=AL.mult)
                nc.sync.dma_start(loc4[:].rearrange("(j p) f -> p j f", p=128),
                                  l4[:])
                nc.gpsimd.collective_compute(
                    "AllGather", AL.bypass, replica_groups=[list(range(8))],
                    ins=[loc4[:]], outs=[gath4[:]])

                # global routing info
                g4 = rp.tile([128, NB, 4], F32)
                nc.sync.dma_start(g4[:], gath4[:].rearrange("(b p) f -> p b f",
                                                            p=128))
                i01 = rp.tile([128, 2, NB], F32)
                g01 = rp.tile([128, 2, NB], F32)
                for s in range(2):
                    nc.vector.tensor_copy(i01[:, s, :], g4[:, :, s])
                    nc.vector.tensor_copy(g01[:, s, :], g4[:, :, 2 + s])

                # one-hots in (e, s, b)-major layout
                oh = rp.tile([128, E, 2, NB], F32)
                for e in range(E):
                    nc.vector.tensor_scalar(out=oh[:, e], in0=i01[:],
                                            scalar1=float(e), scalar2=None,
                                            op0=AL.is_equal)

                # per-(e,s,b) counts via ones-matmul, then segmented scan for
                # exclusive block offsets (slot-1 runs continue slot-0 totals)
                cntA = pr.tile([1, 512], F32, tag="cntA")
                cntB = pr.tile([1, 512], F32, tag="cntB")
                nc.tensor.matmul(cntA[:], lhsT=ones_sb[:], rhs=oh[:, 0:8],
                                 start=True, stop=True)
                nc.tensor.matmul(cntB[:], lhsT=ones_sb[:], rhs=oh[:, 8:16],
                                 start=True, stop=True)
                cnt = rp.tile([1, 2 * E * NB], F32)
                nc.vector.tensor_copy(cnt[:, 0:512], cntA[:])
                nc.vector.tensor_copy(cnt[:, 512:1024], cntB[:])
                msk = rp.tile([1, 2 * E * NB], F32)
                nc.vector.memset(msk[:], 1.0)
                nc.vector.memset(
                    msk[:].rearrange("o (e x) -> o e x", x=2 * NB)[:, :, 2 * NB - 1:],
                    0.0)
                scn = rp.tile([1, 2 * E * NB], F32)
                nc.vector.tensor_tensor_scan(out=scn[:], data0=cnt[:], data1=msk[:],
                                             initial=0.0, op0=AL.add, op1=AL.mult)
                excl = rp.tile([1, 2 * E * NB], F32)
                nc.vector.memset(excl[:, 0:1], 0.0)
                nc.vector.tensor_copy(excl[:, 1:], scn[:, 0:2 * E * NB - 1])
                nc.sync.dma_start(exclb[:, :], excl[0:1, :])
                bc = rp.tile([128, E, 2, NB], F32)
                nc.sync.dma_start(bc[:],
                                  exclb[0:1, :].to_broadcast([128, 2 * E * NB]))

                # in-block inclusive cumsum via tri matmuls
                cuA = pr.tile([128, 512], F32, tag="cuA")
                cuB = pr.tile([128, 512], F32, tag="cuB")
                for e in range(E):
                    for s in range(2):
                        dst = cuA if e < 8 else cuB
                        off = (e % 8) * 2 * NB + s * NB
                        nc.tensor.matmul(dst[:, off:off + NB], lhsT=tri_sb[:],
                                         rhs=oh[:, e, s, :], start=True, stop=True)

                t1 = rp.tile([128, E, 2, NB], F32)
                nc.vector.tensor_tensor(out=t1[:, 0:8], in0=cuA[:].rearrange(
                    "p (e s b) -> p e s b", e=8, s=2), in1=bc[:, 0:8], op=AL.add)
                nc.vector.tensor_tensor(out=t1[:, 8:16], in0=cuB[:].rearrange(
                    "p (e s b) -> p e s b", e=8, s=2), in1=bc[:, 8:16], op=AL.add)
                nc.vector.tensor_tensor(out=t1[:], in0=t1[:], in1=oh[:], op=AL.mult)
                pos = rp.tile([128, 2, NB], F32)
                nc.vector.tensor_reduce(
                    out=pos[:], in_=t1[:].rearrange("p e s b -> p (s b) e"),
                    axis=mybir.AxisListType.X, op=AL.add)
                nc.vector.tensor_scalar_add(pos[:], pos[:], -1.0)

                # local slot offsets: valid -> [0, NSL), invalid -> NSL (dump)
                offc = rp.tile([128, 2, NB], F32)
                m1 = rp.tile([128, 2, NB], F32)
                m2 = rp.tile([128, 2, NB], F32)
                nc.vector.tensor_scalar(m1[:], i01[:], float(C), None, op0=AL.mult)
                nc.vector.tensor_tensor(out=offc[:], in0=m1[:], in1=pos[:],
                                        op=AL.add)
                nc.vector.tensor_scalar_sub(offc[:], offc[:], base_sb[:, 0:1])
                nc.vector.tensor_scalar(m1[:], offc[:], 0.0, None, op0=AL.is_ge)
                nc.vector.tensor_scalar(m2[:], offc[:], float(NSL), None,
                                        op0=AL.is_lt)
                nc.vector.tensor_tensor(out=m1[:], in0=m1[:], in1=m2[:], op=AL.mult)
                nc.vector.tensor_scalar(m2[:], pos[:], float(C), None, op0=AL.is_lt)
                nc.vector.tensor_tensor(out=m1[:], in0=m1[:], in1=m2[:], op=AL.mult)
                nc.vector.tensor_tensor(out=offc[:], in0=offc[:], in1=m1[:],
                                        op=AL.mult)
                nc.vector.tensor_scalar(m2[:], m1[:], float(-NSL), float(NSL),
                                        op0=AL.mult, op1=AL.add)
                nc.vector.tensor_tensor(out=offc[:], in0=offc[:], in1=m2[:],
                                        op=AL.add)

                # payload (tok+1, gate)
                pay = rp.tile([128, 2, NB, 2], F32)
                for s in range(2):
                    nc.vector.tensor_scalar_add(pay[:, s, :, 0], tok_sb[:], 1.0)
                nc.vector.tensor_copy(pay[:, :, :, 1], g01[:])

                # swizzled idx wrap for dma_scatter_add
                offpad = rp.tile([128, 128], F32)
                nc.vector.memset(offpad[:, 64:128], float(NSL))
                nc.vector.tensor_copy(offpad[:, 0:64],
                                      offc[:].rearrange("p s b -> p (s b)"))
                offTp = pr.tile([128, 128], F32, tag="trp")
                nc.tensor.transpose(out=offTp[:], in_=offpad[:], identity=id_sb[:])
                offT = rp.tile([128, 128], I16)
                nc.vector.tensor_copy(offT[:], offTp[:])
                nc.sync.dma_start(
                    idxd[:, :].rearrange("o (q j l) -> (o j) l q", q=16, j=64),
                    offT[0:64, :])
                idxs = rp.tile([128, 512], I16)
                nc.sync.dma_start(
                    idxs[:],
                    idxd[:, :].rearrange("o (q c) -> (o q) c", q=16)[
                        None].to_broadcast([8, 16, 512]))
                payv = pay[:].rearrange("p s b c -> p (s b) c")
                for cc in range(8):
                    nc.gpsimd.dma_scatter_add(
                        out_ap=tab[:, 0:2], in_ap=payv[:, cc * 8:(cc + 1) * 8, :],
                        idxs_ap=idxs[:, cc * 64:(cc + 1) * 64],
                        num_idxs=1024, num_idxs_reg=1024,
                        elem_size=2, elem_step=64, single_packet=False)

                # slot table readback: tokens, gates, gather idx wrap
                tg = rp.tile([128, EL * NCT, 2], F32)
                nc.sync.dma_start(
                    tg[:], tab[0:NSL, 0:2].rearrange("(n p) c -> p n c", p=128))
                tokf = rp.tile([128, EL * NCT], F32)
                nc.vector.tensor_scalar_add(tokf[:], tg[:, :, 0], -1.0)
                nc.vector.tensor_copy(toki[:], tokf[:])
                nc.vector.tensor_copy(gsb[:], tg[:, :, 1])
                m3 = rp.tile([128, EL * NCT], F32)
                nc.vector.tensor_scalar(m3[:], tokf[:], 0.0, float(T + 1),
                                        op0=AL.is_lt, op1=AL.mult)
                tokpad = rp.tile([128, 128], F32)
                nc.vector.memset(tokpad[:, EL * NCT:128], 0.0)
                nc.vector.tensor_tensor(out=tokpad[:, 0:EL * NCT], in0=tokf[:],
                                        in1=m3[:], op=AL.add)
                tokTp = pr.tile([128, 128], F32, tag="trp")
                nc.tensor.transpose(out=tokTp[:], in_=tokpad[:], identity=id_sb[:])
                tokT = rp.tile([128, 128], I16)
                nc.vector.tensor_copy(tokT[:], tokTp[:])
                for e in range(EL):
                    nc.sync.dma_start(
                        tokd[:, e * C:(e + 1) * C].rearrange(
                            "o (q j l) -> (o j) l q", q=16, j=NCT),
                        tokT[e * NCT:(e + 1) * NCT, :])

            # ---------------- expert FFN + combine ----------------
            with tc.tile_pool(name="ffn", bufs=1) as fp, \
                 tc.tile_pool(name="psf", bufs=1, space="PSUM") as pf:
                for e in range(EL):
                    idxe = fp.tile([128, C // 16], I16, tag="idxe", bufs=2)
                    nc.sync.dma_start(
                        idxe[:],
                        tokd[:, e * C:(e + 1) * C].rearrange(
                            "o (q c) -> (o q) c", q=16)[None].to_broadcast(
                            [8, 16, C // 16]))
                    xteb = fp.tile([128, NK, C], BF16, tag="xteb", bufs=1)
                    nc.gpsimd.dma_gather(
                        out_ap=xteb[:], in_ap=xb[:], idxs_ap=idxe[:],
                        num_idxs=C, num_idxs_reg=C, elem_size=D, transpose=True)

                    # mm1 + GELU: hT[f] = gelu(W1[:,f].T @ X.T + b1[f])
                    ht = [fp.tile([128, C], BF16, tag=f"ht{f}", name=f"ht{f}",
                                  bufs=1) for f in range(NF)]
                    for f in range(NF):
                        w1c = fp.tile([128, NK * 128], BF16, tag="w1c", bufs=4)
                        nc.sync.dma_start(w1c[:], w1h[e, f])
                        psA = pf.tile([128, 320], F32, tag="m1", bufs=2)
                        psB = pf.tile([128, 320], F32, tag="m1", bufs=2)
                        for k in range(NK):
                            lw = w1c[:, k * 128:(k + 1) * 128]
                            nc.tensor.matmul(psA[:], lhsT=lw,
                                             rhs=xteb[:, k, 0:320],
                                             start=(k == 0), stop=(k == NK - 1))
                            nc.tensor.matmul(psB[:], lhsT=lw,
                                             rhs=xteb[:, k, 320:640],
                                             start=(k == 0), stop=(k == NK - 1))
                        nc.scalar.activation(ht[f][:, 0:320], psA[:], ACTF.Gelu,
                                             bias=b1_sb[e][:, f:f + 1])
                        nc.scalar.activation(ht[f][:, 320:640], psB[:], ACTF.Gelu,
                                             bias=b1_sb[e][:, f:f + 1])

                    # mm2 + bias + gate-scale; scatter halves at dd 1 and 3
                    yrow = [fp.tile([128, D], F32, tag=f"yrow{ct}", name=f"yrow{ct}",
                                    bufs=1) for ct in range(NCT)]
                    for dd in range(ND):
                        psY = [pf.tile([128, 512], F32, tag=f"m2_{ct}",
                                       name=f"m2_{ct}", bufs=1)
                               for ct in range(NCT)]
                        for f in range(NF):
                            w2c = fp.tile([128, 512], BF16, tag="w2c", bufs=6)
                            nc.sync.dma_start(w2c[:], w2h[e, dd, f])
                            for ct in range(NCT):
                                nc.tensor.matmul(
                                    psY[ct][:],
                                    lhsT=ht[f][:, ct * 128:(ct + 1) * 128],
                                    rhs=w2c[:],
                                    start=(f == 0), stop=(f == NF - 1))
                        for ct in range(NCT):
                            dsl = slice(dd * 512, (dd + 1) * 512)
                            nc.vector.tensor_tensor(out=yrow[ct][:, dsl],
                                                    in0=psY[ct][:],
                                                    in1=b2_sb[e][:, dsl], op=AL.add)
                            nc.vector.tensor_scalar_mul(
                                yrow[ct][:, dsl], yrow[ct][:, dsl],
                                gsb[:, e * NCT + ct:e * NCT + ct + 1])
                            if dd in (1, ND - 1):
                                h0 = 0 if dd == 1 else 1024
                                nc.gpsimd.indirect_dma_start(
                                    out=y[:, :],
                                    out_offset=bass.IndirectOffsetOnAxis(
                                        ap=toki[:, e * NCT + ct:e * NCT + ct + 1],
                                        axis=0),
                                    in_=yrow[ct][:, h0:h0 + 1024], in_offset=None,
                                    element_offset=h0,
                                    bounds_check=T - 1, oob_is_err=False,
                                    compute_op=(AL.bypass if e == 0 else AL.add))

    nc.finalize()
    return nc


def _prep_inputs(x, Wg, W1, b1, W2, b2):
    x = np.asarray(x, np.float32).reshape(T, D)
    xtf = np.asarray(x.T, np.float32)
    xb = np.vstack([x, np.zeros((1, D), np.float32)]).astype(ml_dtypes.bfloat16)
    Wg = np.asarray(Wg, np.float32)
    W1 = np.asarray(W1, np.float32)
    W2 = np.asarray(W2, np.float32)
    b1 = np.asarray(b1, np.float32)
    b2 = np.asarray(b2, np.float32)

    wgp = np.ascontiguousarray(
        Wg.reshape(NK, 128, E).transpose(1, 0, 2).reshape(128, NK * E))
    cb = np.zeros((128, CB_W), np.float32)
    cb[:, CB_TOK:CB_TOK + NB] = (np.arange(NB, dtype=np.float32)[None, :] * 128
                                 + np.arange(128, dtype=np.float32)[:, None])
    cb[:, CB_TRI:CB_TRI + 128] = np.triu(np.ones((128, 128), np.float32))
    cb[:, CB_ID:CB_ID + 128] = np.eye(128, dtype=np.float32)
    cb[:, CB_ONE] = 1.0

    in_maps = []
    for c in range(8):
        el = slice(2 * c, 2 * c + 2)
        w1h = np.ascontiguousarray(
            W1[el].reshape(EL, NK, 128, NF, 128).transpose(0, 3, 2, 1, 4)
        ).astype(ml_dtypes.bfloat16)
        w2h = np.ascontiguousarray(
            W2[el].reshape(EL, NF, 128, ND, 512).transpose(0, 3, 1, 2, 4)
        ).astype(ml_dtypes.bfloat16)
        b1hc = np.ascontiguousarray(b1[el].reshape(EL, NF, 128).transpose(0, 2, 1))
        b2dc = np.ascontiguousarray(b2[el]).astype(ml_dtypes.bfloat16)
        cbc = cb.copy()
        cbc[:, CB_BASE] = float(NSL * c)
        xtloc = np.ascontiguousarray(xtf[:, 512 * c:512 * (c + 1)])
        in_maps.append(dict(xtloc=xtloc, xb=xb, wgp=wgp, w1h=w1h, w2h=w2h,
                            b1h=b1hc, b2d=b2dc, cb=cbc))
    return in_maps


def _run(inputs, trace=False, trace_cores=None):
    if "nc" not in _CACHE:
        _CACHE["nc"] = _build_nc()
    nc = _CACHE["nc"]
    in_maps = _prep_inputs(inputs["x"], inputs["Wg"], inputs["W1"],
                           inputs["b1"], inputs["W2"], inputs["b2"])
    res = run_bass_kernel_spmd(nc, in_maps, list(range(8)), trace=trace,
                               trace_cores=trace_cores)
    y = np.zeros((T, D), np.float64)
    for r in res.results:
        y += r["y"].astype(np.float64)
    y = y.astype(np.float32).reshape(B, S, D)
    return y, res


def kernel(x, Wg, W1, b1, W2, b2):
    y, _ = _run(dict(x=x, Wg=Wg, W1=W1, b1=b1, W2=W2, b2=b2))
    return y


# revision 3
# speedup vs baseline: 1.1453x; 1.1453x over previous
"""MoE block (B=2,S=2048,D=2048,FF=8192,E=16,K=2,C=640) on 8 trn2 cores.

Expert parallelism: 2 experts per core. Gate is data-parallel: each core
computes top-2 + gates for its 512-token stripe in fp32, a small AllGather
replicates [T, 4] = (i0, i1, g0, g1), then every core batch-computes the
capacity positions and builds its 2 experts' slot table with one swizzled
dma_scatter_add. Expert token rows are fetched transposed straight into
SBUF via dma_gather(transpose=True); the FFN runs in bf16 (fp32 accum) and
scatter-adds gate-weighted rows into a per-core partial output that the
host sums (expert-parallel combine/unshard).
"""
import sys
sys.path.insert(0, "/opt/trn_rl_repo")
import numpy as np
import ml_dtypes

import concourse.bass as bass
import concourse.mybir as mybir
import concourse.tile as tile
from concourse import bacc
from concourse.bass_utils import run_bass_kernel_spmd

F32 = mybir.dt.float32
BF16 = mybir.dt.bfloat16
I32 = mybir.dt.int32
I16 = mybir.dt.int16
U32 = mybir.dt.uint32
AL = mybir.AluOpType
ACTF = mybir.ActivationFunctionType

B, S, D, FF, E, K = 2, 2048, 2048, 8192, 16, 2
T = B * S                 # 4096 tokens
C = 640                   # per-expert capacity
NB = T // 128             # 32 token blocks
EL = 2                    # local experts per core
NF = FF // 128            # 64 f-tiles
ND = D // 512             # 4 dd chunks
NCT = C // 128            # 5 capacity tiles per expert
NK = D // 128             # 16 contraction tiles of D
NSL = EL * C              # 1280 local slots
TAB = 1408                # slot table rows (incl dump zone >= NSL)

# consts blob column offsets
CB_TOK = 0
CB_TRI = CB_TOK + NB
CB_ID = CB_TRI + 128
CB_ONE = CB_ID + 128
CB_BASE = CB_ONE + 1
CB_W = CB_BASE + 1

_CACHE = {}


def _build_nc():
    nc = bacc.Bacc(None, target_bir_lowering=False, debug=True)

    xtloc = nc.dram_tensor("xtloc", [D, 512], F32, kind="ExternalInput")
    xb = nc.dram_tensor("xb", [T + 1, D], BF16, kind="ExternalInput")
    wgp = nc.dram_tensor("wgp", [128, NK * E], F32, kind="ExternalInput")
    w1h = nc.dram_tensor("w1h", [EL, NF, 128, NK, 128], BF16, kind="ExternalInput")
    w2h = nc.dram_tensor("w2h", [EL, ND, NF, 128, 512], BF16, kind="ExternalInput")
    b1h = nc.dram_tensor("b1h", [EL, 128, NF], F32, kind="ExternalInput")
    b2d = nc.dram_tensor("b2d", [EL, D], BF16, kind="ExternalInput")
    cb = nc.dram_tensor("cb", [128, CB_W], F32, kind="ExternalInput")

    loc4 = nc.dram_tensor("loc4", [512, 4], F32)
    gath4 = nc.dram_tensor("gath4", [T, 4], F32, addr_space="Shared")
    slq = [nc.dram_tensor(f"slq{q}", [TAB, 2], F32) for q in range(4)]
    tokd = nc.dram_tensor("tokd", [1, NSL], I16)
    exclb = nc.dram_tensor("exclb", [1, 2 * E * NB], F32)

    y = nc.dram_tensor("y", [T, D], F32, kind="ExternalOutput")

    with tile.TileContext(nc) as tc:
        with tc.tile_pool(name="consts", bufs=1) as cp:
            cbs = cp.tile([128, CB_W], F32)
            nc.scalar.dma_start(cbs[:], cb[:])
            tok_sb = cbs[:, CB_TOK:CB_TOK + NB]
            tri_sb = cbs[:, CB_TRI:CB_TRI + 128]
            id_sb = cbs[:, CB_ID:CB_ID + 128]
            ones_sb = cbs[:, CB_ONE:CB_ONE + 1]
            base_sb = cbs[:, CB_BASE:CB_BASE + 1]
            wg_sb = cp.tile([128, NK * E], F32)
            nc.scalar.dma_start(wg_sb[:], wgp[:])
            b1_sb = [cp.tile([128, NF], F32, tag=f"b1_{e}", name=f"b1_{e}")
                     for e in range(EL)]
            b2_sb = [cp.tile([128, D], BF16, tag=f"b2_{e}", name=f"b2_{e}")
                     for e in range(EL)]
            for e in range(EL):
                nc.scalar.dma_start(b1_sb[e][:], b1h[e])
                nc.scalar.dma_start(b2_sb[e][:], b2d[e:e + 1, :].to_broadcast([128, D]))
            gsb = cp.tile([128, EL * NCT], F32)
            toki = cp.tile([128, EL * NCT], I32)

            # ---------------- routing ----------------
            with tc.tile_pool(name="rout", bufs=1) as rp, \
                 tc.tile_pool(name="psr", bufs=1, space="PSUM") as pr:
                # zero-init the slot tables
                zz = rp.tile([128, TAB // 128, 2], F32)
                nc.vector.memset(zz[:], 0.0)
                for q in range(4):
                    nc.sync.dma_start(
                        slq[q][:].rearrange("(n p) c -> p n c", p=128), zz[:])

                # gate for the local 512-token stripe (fp32, data-parallel)
                xtg = rp.tile([128, NK, 512], F32)
                nc.sync.dma_start(xtg[:], xtloc[:].rearrange("(k p) c -> p k c",
                                                             p=128))
                locsb = rp.tile([128, 4, E], F32)
                for j in range(4):
                    lg = pr.tile([128, E], F32, tag="lg", bufs=2)
                    for k in range(NK):
                        nc.tensor.matmul(lg[:], lhsT=xtg[:, k, j * 128:(j + 1) * 128],
                                         rhs=wg_sb[:, k * E:(k + 1) * E],
                                         start=(k == 0), stop=(k == NK - 1))
                    nc.vector.tensor_copy(locsb[:, j, :], lg[:])

                # local top-2 + gates
                mx = rp.tile([128, 4, 8], F32)
                mi = rp.tile([128, 4, 8], U32)
                dte = rp.tile([128, 4], F32)
                exd = rp.tile([128, 4], F32)
                den = rp.tile([128, 4], F32)
                g0l = rp.tile([128, 4], F32)
                l4 = rp.tile([128, 4, 4], F32)
                for j in range(4):
                    nc.vector.max(out=mx[:, j, :], in_=locsb[:, j, :])
                    nc.vector.max_index(out=mi[:, j, :], in_max=mx[:, j, :],
                                        in_values=locsb[:, j, :])
                    nc.vector.tensor_tensor(out=dte[:, j:j + 1], in0=mx[:, j, 1:2],
                                            in1=mx[:, j, 0:1], op=AL.subtract)
                nc.scalar.activation(exd[:], dte[:], ACTF.Exp)
                nc.vector.tensor_scalar_add(den[:], exd[:], 1.0)
                nc.vector.reciprocal(g0l[:], den[:])
                nc.vector.tensor_copy(l4[:, :, 0], mi[:, :, 0])
                nc.vector.tensor_copy(l4[:, :, 1], mi[:, :, 1])
                nc.vector.tensor_copy(l4[:, :, 2], g0l[:])
                nc.vector.tensor_tensor(out=l4[:, :, 3], in0=exd[:], in1=g0l[:],
                                        op=AL.mult)
                nc.sync.dma_start(loc4[:].rearrange("(j p) f -> p j f", p=128),
                                  l4[:])
                nc.gpsimd.collective_compute(
                    "AllGather", AL.bypass, replica_groups=[list(range(8))],
                    ins=[loc4[:]], outs=[gath4[:]])

                # global routing info
                g4 = rp.tile([128, NB, 4], F32)
                nc.sync.dma_start(g4[:], gath4[:].rearrange("(b p) f -> p b f",
                                                            p=128))
                i01 = rp.tile([128, 2, NB], F32)
                g01 = rp.tile([128, 2, NB], F32)
                for s in range(2):
                    nc.vector.tensor_copy(i01[:, s, :], g4[:, :, s])
                    nc.vector.tensor_copy(g01[:, s, :], g4[:, :, 2 + s])

                # one-hots in (e, s, b)-major layout
                oh = rp.tile([128, E, 2, NB], F32)
                for e in range(E):
                    nc.vector.tensor_scalar(out=oh[:, e], in0=i01[:],
                                            scalar1=float(e), scalar2=None,
                                            op0=AL.is_equal)

                # per-(e,s,b) counts via ones-matmul, then segmented scan for
                # exclusive block offsets (slot-1 runs continue slot-0 totals)
                cntA = pr.tile([1, 512], F32, tag="cntA")
                cntB = pr.tile([1, 512], F32, tag="cntB")
                nc.tensor.matmul(cntA[:], lhsT=ones_sb[:], rhs=oh[:, 0:8],
                                 start=True, stop=True)
                nc.tensor.matmul(cntB[:], lhsT=ones_sb[:], rhs=oh[:, 8:16],
                                 start=True, stop=True)
                cnt = rp.tile([1, 2 * E * NB], F32)
                nc.vector.tensor_copy(cnt[:, 0:512], cntA[:])
                nc.vector.tensor_copy(cnt[:, 512:1024], cntB[:])
                msk = rp.tile([1, 2 * E * NB], F32)
                nc.vector.memset(msk[:], 1.0)
                nc.vector.memset(
                    msk[:].rearrange("o (e x) -> o e x", x=2 * NB)[:, :, 2 * NB - 1:],
                    0.0)
                scn = rp.tile([1, 2 * E * NB], F32)
                nc.vector.tensor_tensor_scan(out=scn[:], data0=cnt[:], data1=msk[:],
                                             initial=0.0, op0=AL.add, op1=AL.mult)
                excl = rp.tile([1, 2 * E * NB], F32)
                nc.vector.memset(excl[:, 0:1], 0.0)
                nc.vector.tensor_copy(excl[:, 1:], scn[:, 0:2 * E * NB - 1])
                nc.sync.dma_start(exclb[:, :], excl[0:1, :])
                bc = rp.tile([128, E, 2, NB], F32)
                nc.sync.dma_start(bc[:],
                                  exclb[0:1, :].to_broadcast([128, 2 * E * NB]))

                # in-block inclusive cumsum via tri matmuls
                cuA = pr.tile([128, 512], F32, tag="cuA")
                cuB = pr.tile([128, 512], F32, tag="cuB")
                for e in range(E):
                    for s in range(2):
                        dst = cuA if e < 8 else cuB
                        off = (e % 8) * 2 * NB + s * NB
                        nc.tensor.matmul(dst[:, off:off + NB], lhsT=tri_sb[:],
                                         rhs=oh[:, e, s, :], start=True, stop=True)

                t1 = rp.tile([128, E, 2, NB], F32)
                nc.vector.tensor_tensor(out=t1[:, 0:8], in0=cuA[:].rearrange(
                    "p (e s b) -> p e s b", e=8, s=2), in1=bc[:, 0:8], op=AL.add)
                nc.vector.tensor_tensor(out=t1[:, 8:16], in0=cuB[:].rearrange(
                    "p (e s b) -> p e s b", e=8, s=2), in1=bc[:, 8:16], op=AL.add)
                nc.vector.tensor_tensor(out=t1[:], in0=t1[:], in1=oh[:], op=AL.mult)
                pos = rp.tile([128, 2, NB], F32)
                nc.vector.tensor_reduce(
                    out=pos[:], in_=t1[:].rearrange("p e s b -> p (s b) e"),
                    axis=mybir.AxisListType.X, op=AL.add)
                nc.vector.tensor_scalar_add(pos[:], pos[:], -1.0)

                # local slot offsets: valid -> [0, NSL), invalid -> NSL (dump)
                offc = rp.tile([128, 2, NB], F32)
                m1 = rp.tile([128, 2, NB], F32)
                m2 = rp.tile([128, 2, NB], F32)
                nc.vector.tensor_scalar(m1[:], i01[:], float(C), None, op0=AL.mult)
                nc.vector.tensor_tensor(out=offc[:], in0=m1[:], in1=pos[:],
                                        op=AL.add)
                nc.vector.tensor_scalar_sub(offc[:], offc[:], base_sb[:, 0:1])
                nc.vector.tensor_scalar(m1[:], offc[:], 0.0, None, op0=AL.is_ge)
                nc.vector.tensor_scalar(m2[:], offc[:], float(NSL), None,
                                        op0=AL.is_lt)
                nc.vector.tensor_tensor(out=m1[:], in0=m1[:], in1=m2[:], op=AL.mult)
                nc.vector.tensor_scalar(m2[:], pos[:], float(C), None, op0=AL.is_lt)
                nc.vector.tensor_tensor(out=m1[:], in0=m1[:], in1=m2[:], op=AL.mult)
                nc.vector.tensor_tensor(out=offc[:], in0=offc[:], in1=m1[:],
                                        op=AL.mult)
                nc.vector.tensor_scalar(m2[:], m1[:], float(-NSL), float(NSL),
                                        op0=AL.mult, op1=AL.add)
                nc.vector.tensor_tensor(out=offc[:], in0=offc[:], in1=m2[:],
                                        op=AL.add)

                # payload (tok+1, gate)
                pay = rp.tile([128, 2, NB, 2], F32)
                for s in range(2):
                    nc.vector.tensor_scalar_add(pay[:, s, :, 0], tok_sb[:], 1.0)
                nc.vector.tensor_copy(pay[:, :, :, 1], g01[:])

                # scatter (tok+1, gate) rows into 4 round-robin tables
                offi = rp.tile([128, 2, NB], I32)
                nc.vector.tensor_copy(offi[:], offc[:])
                breg = nc.gpsimd.snap(NSL)
                for b in range(NB):
                    for s in range(2):
                        q = (2 * b + s) % 4
                        nc.gpsimd.indirect_dma_start(
                            out=slq[q][:, :],
                            out_offset=bass.IndirectOffsetOnAxis(
                                ap=offi[:, s, b:b + 1], axis=0),
                            in_=pay[:, s, b, :], in_offset=None,
                            bounds_check=breg, oob_is_err=False)

                # slot table readback + merge (empty rows are (0,0); each
                # written slot lives in exactly one table, so merge = sum)
                tgq = [rp.tile([128, EL * NCT, 2], F32, tag=f"tgq{q}",
                               name=f"tgq{q}") for q in range(4)]
                for q in range(4):
                    nc.sync.dma_start(
                        tgq[q][:],
                        slq[q][0:NSL, :].rearrange("(n p) c -> p n c", p=128))
                nc.vector.tensor_tensor(out=tgq[0][:], in0=tgq[0][:], in1=tgq[1][:],
                                        op=AL.add)
                nc.vector.tensor_tensor(out=tgq[2][:], in0=tgq[2][:], in1=tgq[3][:],
                                        op=AL.add)
                tg = rp.tile([128, EL * NCT, 2], F32)
                nc.vector.tensor_tensor(out=tg[:], in0=tgq[0][:], in1=tgq[2][:],
                                        op=AL.add)
                tokf = rp.tile([128, EL * NCT], F32)
                nc.vector.tensor_scalar_add(tokf[:], tg[:, :, 0], -1.0)
                nc.vector.tensor_copy(toki[:], tokf[:])
                nc.vector.tensor_copy(gsb[:], tg[:, :, 1])
                m3 = rp.tile([128, EL * NCT], F32)
                nc.vector.tensor_scalar(m3[:], tokf[:], 0.0, float(T + 1),
                                        op0=AL.is_lt, op1=AL.mult)
                tokpad = rp.tile([128, 128], F32)
                nc.vector.memset(tokpad[:, EL * NCT:128], 0.0)
                nc.vector.tensor_tensor(out=tokpad[:, 0:EL * NCT], in0=tokf[:],
                                        in1=m3[:], op=AL.add)
                tokTp = pr.tile([128, 128], F32, tag="trp")
                nc.tensor.transpose(out=tokTp[:], in_=tokpad[:], identity=id_sb[:])
                tokT = rp.tile([128, 128], I16)
                nc.vector.tensor_copy(tokT[:], tokTp[:])
                for e in range(EL):
                    nc.sync.dma_start(
                        tokd[:, e * C:(e + 1) * C].rearrange(
                            "o (q j l) -> (o j) l q", q=16, j=NCT),
                        tokT[e * NCT:(e + 1) * NCT, :])

            # ---------------- expert FFN + combine ----------------
            with tc.tile_pool(name="ffn", bufs=1) as fp, \
                 tc.tile_pool(name="psf", bufs=1, space="PSUM") as pf:
                for e in range(EL):
                    idxe = fp.tile([128, C // 16], I16, tag="idxe", bufs=2)
                    nc.sync.dma_start(
                        idxe[:],
                        tokd[:, e * C:(e + 1) * C].rearrange(
                            "o (q c) -> (o q) c", q=16)[None].to_broadcast(
                            [8, 16, C // 16]))
                    xteb = fp.tile([128, NK, C], BF16, tag="xteb", bufs=1)
                    nc.gpsimd.dma_gather(
                        out_ap=xteb[:], in_ap=xb[:], idxs_ap=idxe[:],
                        num_idxs=C, num_idxs_reg=C, elem_size=D, transpose=True)

                    # mm1 + GELU: hT[f] = gelu(W1[:,f].T @ X.T + b1[f])
                    ht = [fp.tile([128, C], BF16, tag=f"ht{f}", name=f"ht{f}",
                                  bufs=1) for f in range(NF)]
                    for f in range(NF):
                        w1c = fp.tile([128, NK * 128], BF16, tag="w1c", bufs=4)
                        nc.sync.dma_start(w1c[:], w1h[e, f])
                        psA = pf.tile([128, 320], F32, tag="m1", bufs=2)
                        psB = pf.tile([128, 320], F32, tag="m1", bufs=2)
                        for k in range(NK):
                            lw = w1c[:, k * 128:(k + 1) * 128]
                            nc.tensor.matmul(psA[:], lhsT=lw,
                                             rhs=xteb[:, k, 0:320],
                                             start=(k == 0), stop=(k == NK - 1))
                            nc.tensor.matmul(psB[:], lhsT=lw,
                                             rhs=xteb[:, k, 320:640],
                                             start=(k == 0), stop=(k == NK - 1))
                        nc.scalar.activation(ht[f][:, 0:320], psA[:], ACTF.Gelu,
                                             bias=b1_sb[e][:, f:f + 1])
                        nc.scalar.activation(ht[f][:, 320:640], psB[:], ACTF.Gelu,
                                             bias=b1_sb[e][:, f:f + 1])

                    # mm2 + bias + gate-scale; scatter halves at dd 1 and 3
                    yrow = [fp.tile([128, D], F32, tag=f"yrow{ct}", name=f"yrow{ct}",
                                    bufs=1) for ct in range(NCT)]
                    for dd in range(ND):
                        psY = [pf.tile([128, 512], F32, tag=f"m2_{ct}",
                                       name=f"m2_{ct}", bufs=1)
                               for ct in range(NCT)]
                        for f in range(NF):
                            w2c = fp.tile([128, 512], BF16, tag="w2c", bufs=6)
                            nc.sync.dma_start(w2c[:], w2h[e, dd, f])
                            for ct in range(NCT):
                                nc.tensor.matmul(
                                    psY[ct][:],
                                    lhsT=ht[f][:, ct * 128:(ct + 1) * 128],
                                    rhs=w2c[:],
                                    start=(f == 0), stop=(f == NF - 1))
                        for ct in range(NCT):
                            dsl = slice(dd * 512, (dd + 1) * 512)
                            nc.vector.tensor_tensor(out=yrow[ct][:, dsl],
                                                    in0=psY[ct][:],
                                                    in1=b2_sb[e][:, dsl], op=AL.add)
                            nc.vector.tensor_scalar_mul(
                                yrow[ct][:, dsl], yrow[ct][:, dsl],
                                gsb[:, e * NCT + ct:e * NCT + ct + 1])
                            if dd in (1, ND - 1):
                                h0 = 0 if dd == 1 else 1024
                                nc.gpsimd.indirect_dma_start(
                                    out=y[:, :],
                                    out_offset=bass.IndirectOffsetOnAxis(
                                        ap=toki[:, e * NCT + ct:e * NCT + ct + 1],
                                        axis=0),
                                    in_=yrow[ct][:, h0:h0 + 1024], in_offset=None,
                                    element_offset=h0,
                                    bounds_check=T - 1, oob_is_err=False,
                                    compute_op=(AL.bypass if e == 0 else AL.add))

    nc.finalize()
    return nc


def _prep_inputs(x, Wg, W1, b1, W2, b2):
    x = np.asarray(x, np.float32).reshape(T, D)
    xtf = np.asarray(x.T, np.float32)
    xb = np.vstack([x, np.zeros((1, D), np.float32)]).astype(ml_dtypes.bfloat16)
    Wg = np.asarray(Wg, np.float32)
    W1 = np.asarray(W1, np.float32)
    W2 = np.asarray(W2, np.float32)
    b1 = np.asarray(b1, np.float32)
    b2 = np.asarray(b2, np.float32)

    wgp = np.ascontiguousarray(
        Wg.reshape(NK, 128, E).transpose(1, 0, 2).reshape(128, NK * E))
    cb = np.zeros((128, CB_W), np.float32)
    cb[:, CB_TOK:CB_TOK + NB] = (np.arange(NB, dtype=np.float32)[None, :] * 128
                                 + np.arange(128, dtype=np.float32)[:, None])
    cb[:, CB_TRI:CB_TRI + 128] = np.triu(np.ones((128, 128), np.float32))
    cb[:, CB_ID:CB_ID + 128] = np.eye(128, dtype=np.float32)
    cb[:, CB_ONE] = 1.0

    in_maps = []
    for c in range(8):
        el = slice(2 * c, 2 * c + 2)
        w1h = np.ascontiguousarray(
            W1[el].reshape(EL, NK, 128, NF, 128).transpose(0, 3, 2, 1, 4)
        ).astype(ml_dtypes.bfloat16)
        w2h = np.ascontiguousarray(
            W2[el].reshape(EL, NF, 128, ND, 512).transpose(0, 3, 1, 2, 4)
        ).astype(ml_dtypes.bfloat16)
        b1hc = np.ascontiguousarray(b1[el].reshape(EL, NF, 128).transpose(0, 2, 1))
        b2dc = np.ascontiguousarray(b2[el]).astype(ml_dtypes.bfloat16)
        cbc = cb.copy()
        cbc[:, CB_BASE] = float(NSL * c)
        xtloc = np.ascontiguousarray(xtf[:, 512 * c:512 * (c + 1)])
        in_maps.append(dict(xtloc=xtloc, xb=xb, wgp=wgp, w1h=w1h, w2h=w2h,
                            b1h=b1hc, b2d=b2dc, cb=cbc))
    return in_maps


def _run(inputs, trace=False, trace_cores=None):
    if "nc" not in _CACHE:
        _CACHE["nc"] = _build_nc()
    nc = _CACHE["nc"]
    in_maps = _prep_inputs(inputs["x"], inputs["Wg"], inputs["W1"],
                           inputs["b1"], inputs["W2"], inputs["b2"])
    res = run_bass_kernel_spmd(nc, in_maps, list(range(8)), trace=trace,
                               trace_cores=trace_cores)
    y = np.zeros((T, D), np.float64)
    for r in res.results:
        y += r["y"].astype(np.float64)
    y = y.astype(np.float32).reshape(B, S, D)
    return y, res


def kernel(x, Wg, W1, b1, W2, b2):
    y, _ = _run(dict(x=x, Wg=Wg, W1=W1, b1=b1, W2=W2, b2=b2))
    return y


# revision 4
# speedup vs baseline: 1.7323x; 1.5126x over previous
"""MoE block (B=2,S=2048,D=2048,FF=8192,E=16,K=2,C=640) on 8 trn2 cores.

Expert parallelism: 2 experts per core. Gate is data-parallel: each core
computes top-2 + gates for its 512-token stripe in fp32, a small AllGather
replicates [T, 4] = (i0, i1, g0, g1), then every core batch-computes the
capacity positions and builds its 2 experts' slot table with one swizzled
dma_scatter_add. Expert token rows are fetched transposed straight into
SBUF via dma_gather(transpose=True); the FFN runs in bf16 (fp32 accum) and
scatter-adds gate-weighted rows into a per-core partial output that the
host sums (expert-parallel combine/unshard).
"""
import sys
sys.path.insert(0, "/opt/trn_rl_repo")
import numpy as np
import ml_dtypes

import concourse.bass as bass
import concourse.mybir as mybir
import concourse.tile as tile
from concourse import bacc
from concourse.bass_utils import run_bass_kernel_spmd

F32 = mybir.dt.float32
BF16 = mybir.dt.bfloat16
I32 = mybir.dt.int32
I16 = mybir.dt.int16
U32 = mybir.dt.uint32
AL = mybir.AluOpType
ACTF = mybir.ActivationFunctionType

B, S, D, FF, E, K = 2, 2048, 2048, 8192, 16, 2
T = B * S                 # 4096 tokens
C = 640                   # per-expert capacity
NB = T // 128             # 32 token blocks
EL = 2                    # local experts per core
NF = FF // 128            # 64 f-tiles
ND = D // 512             # 4 dd chunks
NCT = C // 128            # 5 capacity tiles per expert
NK = D // 128             # 16 contraction tiles of D
NSL = EL * C              # 1280 local slots
TAB = 1408                # slot table rows (incl dump zone >= NSL)

# consts blob column offsets
CB_TOK = 0
CB_TRI = CB_TOK + NB
CB_ID = CB_TRI + 128
CB_ONE = CB_ID + 128
CB_BASE = CB_ONE + 1
CB_W = CB_BASE + 1

_CACHE = {}


def _build_nc():
    nc = bacc.Bacc(None, target_bir_lowering=False, debug=True)

    xtloc = nc.dram_tensor("xtloc", [D, 512], F32, kind="ExternalInput")
    xb = nc.dram_tensor("xb", [T + 1, D], BF16, kind="ExternalInput")
    wgp = nc.dram_tensor("wgp", [128, NK * E], F32, kind="ExternalInput")
    w1h = nc.dram_tensor("w1h", [EL, NF, 128, NK, 128], BF16, kind="ExternalInput")
    w2h = nc.dram_tensor("w2h", [EL, ND, NF, 128, 512], BF16, kind="ExternalInput")
    b1h = nc.dram_tensor("b1h", [EL, 128, NF], F32, kind="ExternalInput")
    b2d = nc.dram_tensor("b2d", [EL, D], BF16, kind="ExternalInput")
    cb = nc.dram_tensor("cb", [128, CB_W], F32, kind="ExternalInput")

    loc4 = nc.dram_tensor("loc4", [512, 4], F32)
    gath4 = nc.dram_tensor("gath4", [T, 4], F32, addr_space="Shared")
    slq = [nc.dram_tensor(f"slq{q}", [TAB, 2], F32) for q in range(4)]
    tokd = nc.dram_tensor("tokd", [1, NSL], I16)
    exclb = nc.dram_tensor("exclb", [1, 2 * E * NB], F32)

    y = nc.dram_tensor("y", [T, D], F32, kind="ExternalOutput")

    with tile.TileContext(nc) as tc:
        with tc.tile_pool(name="consts", bufs=1) as cp:
            cbs = cp.tile([128, CB_W], F32)
            nc.scalar.dma_start(cbs[:], cb[:])
            tok_sb = cbs[:, CB_TOK:CB_TOK + NB]
            tri_sb = cbs[:, CB_TRI:CB_TRI + 128]
            id_sb = cbs[:, CB_ID:CB_ID + 128]
            ones_sb = cbs[:, CB_ONE:CB_ONE + 1]
            base_sb = cbs[:, CB_BASE:CB_BASE + 1]
            wg_sb = cp.tile([128, NK * E], F32)
            nc.scalar.dma_start(wg_sb[:], wgp[:])
            b1_sb = [cp.tile([128, NF], F32, tag=f"b1_{e}", name=f"b1_{e}")
                     for e in range(EL)]
            b2_sb = [cp.tile([128, D], BF16, tag=f"b2_{e}", name=f"b2_{e}")
                     for e in range(EL)]
            for e in range(EL):
                nc.scalar.dma_start(b1_sb[e][:], b1h[e])
                nc.scalar.dma_start(b2_sb[e][:], b2d[e:e + 1, :].to_broadcast([128, D]))
            gsb = cp.tile([128, EL * NCT], F32)
            toki = cp.tile([128, EL * NCT], I32)

            # ---------------- routing ----------------
            with tc.tile_pool(name="rout", bufs=1) as rp, \
                 tc.tile_pool(name="psr", bufs=1, space="PSUM") as pr:
                # zero-init the slot tables
                zz = rp.tile([128, TAB // 128, 2], F32)
                nc.vector.memset(zz[:], 0.0)
                for q in range(4):
                    nc.sync.dma_start(
                        slq[q][:].rearrange("(n p) c -> p n c", p=128), zz[:])

                # gate for the local 512-token stripe (fp32, data-parallel)
                xtg = rp.tile([128, NK, 512], F32)
                nc.sync.dma_start(xtg[:], xtloc[:].rearrange("(k p) c -> p k c",
                                                             p=128))
                locsb = rp.tile([128, 4, E], F32)
                for j in range(4):
                    lg = pr.tile([128, E], F32, tag="lg", bufs=2)
                    for k in range(NK):
                        nc.tensor.matmul(lg[:], lhsT=xtg[:, k, j * 128:(j + 1) * 128],
                                         rhs=wg_sb[:, k * E:(k + 1) * E],
                                         start=(k == 0), stop=(k == NK - 1))
                    nc.vector.tensor_copy(locsb[:, j, :], lg[:])

                # local top-2 + gates
                mx = rp.tile([128, 4, 8], F32)
                mi = rp.tile([128, 4, 8], U32)
                dte = rp.tile([128, 4], F32)
                exd = rp.tile([128, 4], F32)
                den = rp.tile([128, 4], F32)
                g0l = rp.tile([128, 4], F32)
                l4 = rp.tile([128, 4, 4], F32)
                for j in range(4):
                    nc.vector.max(out=mx[:, j, :], in_=locsb[:, j, :])
                    nc.vector.max_index(out=mi[:, j, :], in_max=mx[:, j, :],
                                        in_values=locsb[:, j, :])
                    nc.vector.tensor_tensor(out=dte[:, j:j + 1], in0=mx[:, j, 1:2],
                                            in1=mx[:, j, 0:1], op=AL.subtract)
                nc.scalar.activation(exd[:], dte[:], ACTF.Exp)
                nc.vector.tensor_scalar_add(den[:], exd[:], 1.0)
                nc.vector.reciprocal(g0l[:], den[:])
                nc.vector.tensor_copy(l4[:, :, 0], mi[:, :, 0])
                nc.vector.tensor_copy(l4[:, :, 1], mi[:, :, 1])
                nc.vector.tensor_copy(l4[:, :, 2], g0l[:])
                nc.vector.tensor_tensor(out=l4[:, :, 3], in0=exd[:], in1=g0l[:],
                                        op=AL.mult)
                nc.sync.dma_start(loc4[:].rearrange("(j p) f -> p j f", p=128),
                                  l4[:])
                nc.gpsimd.collective_compute(
                    "AllGather", AL.bypass, replica_groups=[list(range(8))],
                    ins=[loc4[:]], outs=[gath4[:]])

                # global routing info
                g4 = rp.tile([128, NB, 4], F32)
                nc.sync.dma_start(g4[:], gath4[:].rearrange("(b p) f -> p b f",
                                                            p=128))
                i01 = rp.tile([128, 2, NB], F32)
                g01 = rp.tile([128, 2, NB], F32)
                for s in range(2):
                    nc.vector.tensor_copy(i01[:, s, :], g4[:, :, s])
                    nc.vector.tensor_copy(g01[:, s, :], g4[:, :, 2 + s])

                # one-hots in (e, s, b)-major layout
                oh = rp.tile([128, E, 2, NB], F32)
                for e in range(E):
                    nc.vector.tensor_scalar(out=oh[:, e], in0=i01[:],
                                            scalar1=float(e), scalar2=None,
                                            op0=AL.is_equal)

                # per-(e,s,b) counts via ones-matmul, then segmented scan for
                # exclusive block offsets (slot-1 runs continue slot-0 totals)
                cntA = pr.tile([1, 512], F32, tag="cntA")
                cntB = pr.tile([1, 512], F32, tag="cntB")
                nc.tensor.matmul(cntA[:], lhsT=ones_sb[:], rhs=oh[:, 0:8],
                                 start=True, stop=True)
                nc.tensor.matmul(cntB[:], lhsT=ones_sb[:], rhs=oh[:, 8:16],
                                 start=True, stop=True)
                cnt = rp.tile([1, 2 * E * NB], F32)
                nc.vector.tensor_copy(cnt[:, 0:512], cntA[:])
                nc.vector.tensor_copy(cnt[:, 512:1024], cntB[:])
                msk = rp.tile([1, 2 * E * NB], F32)
                nc.vector.memset(msk[:], 1.0)
                nc.vector.memset(
                    msk[:].rearrange("o (e x) -> o e x", x=2 * NB)[:, :, 2 * NB - 1:],
                    0.0)
                scn = rp.tile([1, 2 * E * NB], F32)
                nc.vector.tensor_tensor_scan(out=scn[:], data0=cnt[:], data1=msk[:],
                                             initial=0.0, op0=AL.add, op1=AL.mult)
                excl = rp.tile([1, 2 * E * NB], F32)
                nc.vector.memset(excl[:, 0:1], 0.0)
                nc.vector.tensor_copy(excl[:, 1:], scn[:, 0:2 * E * NB - 1])
                nc.sync.dma_start(exclb[:, :], excl[0:1, :])
                bc = rp.tile([128, E, 2, NB], F32)
                nc.sync.dma_start(bc[:],
                                  exclb[0:1, :].to_broadcast([128, 2 * E * NB]))

                # in-block inclusive cumsum via tri matmuls
                cuA = pr.tile([128, 512], F32, tag="cuA")
                cuB = pr.tile([128, 512], F32, tag="cuB")
                for e in range(E):
                    for s in range(2):
                        dst = cuA if e < 8 else cuB
                        off = (e % 8) * 2 * NB + s * NB
                        nc.tensor.matmul(dst[:, off:off + NB], lhsT=tri_sb[:],
                                         rhs=oh[:, e, s, :], start=True, stop=True)

                t1 = rp.tile([128, E, 2, NB], F32)
                nc.vector.tensor_tensor(out=t1[:, 0:8], in0=cuA[:].rearrange(
                    "p (e s b) -> p e s b", e=8, s=2), in1=bc[:, 0:8], op=AL.add)
                nc.vector.tensor_tensor(out=t1[:, 8:16], in0=cuB[:].rearrange(
                    "p (e s b) -> p e s b", e=8, s=2), in1=bc[:, 8:16], op=AL.add)
                nc.vector.tensor_tensor(out=t1[:], in0=t1[:], in1=oh[:], op=AL.mult)
                pos = rp.tile([128, 2, NB], F32)
                nc.vector.tensor_reduce(
                    out=pos[:], in_=t1[:].rearrange("p e s b -> p (s b) e"),
                    axis=mybir.AxisListType.X, op=AL.add)
                nc.vector.tensor_scalar_add(pos[:], pos[:], -1.0)

                # local slot offsets: valid -> [0, NSL), invalid -> NSL (dump)
                offc = rp.tile([128, 2, NB], F32)
                m1 = rp.tile([128, 2, NB], F32)
                m2 = rp.tile([128, 2, NB], F32)
                nc.vector.tensor_scalar(m1[:], i01[:], float(C), None, op0=AL.mult)
                nc.vector.tensor_tensor(out=offc[:], in0=m1[:], in1=pos[:],
                                        op=AL.add)
                nc.vector.tensor_scalar_sub(offc[:], offc[:], base_sb[:, 0:1])
                nc.vector.tensor_scalar(m1[:], offc[:], 0.0, None, op0=AL.is_ge)
                nc.vector.tensor_scalar(m2[:], offc[:], float(NSL), None,
                                        op0=AL.is_lt)
                nc.vector.tensor_tensor(out=m1[:], in0=m1[:], in1=m2[:], op=AL.mult)
                nc.vector.tensor_scalar(m2[:], pos[:], float(C), None, op0=AL.is_lt)
                nc.vector.tensor_tensor(out=m1[:], in0=m1[:], in1=m2[:], op=AL.mult)
                nc.vector.tensor_tensor(out=offc[:], in0=offc[:], in1=m1[:],
                                        op=AL.mult)
                nc.vector.tensor_scalar(m2[:], m1[:], -2.0e9, 2.0e9,
                                        op0=AL.mult, op1=AL.add)
                nc.vector.tensor_tensor(out=offc[:], in0=offc[:], in1=m2[:],
                                        op=AL.add)

                # payload (tok+1, gate)
                pay = rp.tile([128, 2, NB, 2], F32)
                for s in range(2):
                    nc.vector.tensor_scalar_add(pay[:, s, :, 0], tok_sb[:], 1.0)
                nc.vector.tensor_copy(pay[:, :, :, 1], g01[:])

                # scatter (tok+1, gate) rows into 4 round-robin tables
                offi = rp.tile([128, 2, NB], I32)
                nc.vector.tensor_copy(offi[:], offc[:])
                breg = nc.gpsimd.snap(NSL - 1)
                for b in range(NB):
                    for s in range(2):
                        q = (2 * b + s) % 4
                        nc.gpsimd.indirect_dma_start(
                            out=slq[q][:, :],
                            out_offset=bass.IndirectOffsetOnAxis(
                                ap=offi[:, s, b:b + 1], axis=0),
                            in_=pay[:, s, b, :], in_offset=None,
                            bounds_check=breg, oob_is_err=False)

                # slot table readback + merge (empty rows are (0,0); each
                # written slot lives in exactly one table, so merge = sum)
                tgq = [rp.tile([128, EL * NCT, 2], F32, tag=f"tgq{q}",
                               name=f"tgq{q}") for q in range(4)]
                for q in range(4):
                    nc.sync.dma_start(
                        tgq[q][:],
                        slq[q][0:NSL, :].rearrange("(n p) c -> p n c", p=128))
                nc.vector.tensor_tensor(out=tgq[0][:], in0=tgq[0][:], in1=tgq[1][:],
                                        op=AL.add)
                nc.vector.tensor_tensor(out=tgq[2][:], in0=tgq[2][:], in1=tgq[3][:],
                                        op=AL.add)
                tg = rp.tile([128, EL * NCT, 2], F32)
                nc.vector.tensor_tensor(out=tg[:], in0=tgq[0][:], in1=tgq[2][:],
                                        op=AL.add)
                tokf = rp.tile([128, EL * NCT], F32)
                nc.vector.tensor_scalar_add(tokf[:], tg[:, :, 0], -1.0)
                nc.vector.tensor_copy(toki[:], tokf[:])
                nc.vector.tensor_copy(gsb[:], tg[:, :, 1])
                m3 = rp.tile([128, EL * NCT], F32)
                nc.vector.tensor_scalar(m3[:], tokf[:], 0.0, float(T + 1),
                                        op0=AL.is_lt, op1=AL.mult)
                tokpad = rp.tile([128, 128], F32)
                nc.vector.memset(tokpad[:, EL * NCT:128], 0.0)
                nc.vector.tensor_tensor(out=tokpad[:, 0:EL * NCT], in0=tokf[:],
                                        in1=m3[:], op=AL.add)
                tokTp = pr.tile([128, 128], F32, tag="trp")
                nc.tensor.transpose(out=tokTp[:], in_=tokpad[:], identity=id_sb[:])
                tokT = rp.tile([128, 128], I16)
                nc.vector.tensor_copy(tokT[:], tokTp[:])
                for e in range(EL):
                    nc.sync.dma_start(
                        tokd[:, e * C:(e + 1) * C].rearrange(
                            "o (q j l) -> (o j) l q", q=16, j=NCT),
                        tokT[e * NCT:(e + 1) * NCT, :])

            # ---------------- expert FFN + combine ----------------
            with tc.tile_pool(name="ffn", bufs=1) as fp, \
                 tc.tile_pool(name="psf", bufs=1, space="PSUM") as pf:
                for e in range(EL):
                    idxe = fp.tile([128, C // 16], I16, tag="idxe", bufs=2)
                    nc.sync.dma_start(
                        idxe[:],
                        tokd[:, e * C:(e + 1) * C].rearrange(
                            "o (q c) -> (o q) c", q=16)[None].to_broadcast(
                            [8, 16, C // 16]))
                    xteb = fp.tile([128, NK, C], BF16, tag="xteb", bufs=1)
                    nc.gpsimd.dma_gather(
                        out_ap=xteb[:], in_ap=xb[:], idxs_ap=idxe[:],
                        num_idxs=C, num_idxs_reg=C, elem_size=D, transpose=True)

                    # mm1 + GELU: hT[f] = gelu(W1[:,f].T @ X.T + b1[f])
                    ht = [fp.tile([128, C], BF16, tag=f"ht{f}", name=f"ht{f}",
                                  bufs=1) for f in range(NF)]
                    for f in range(NF):
                        w1c = fp.tile([128, NK * 128], BF16, tag="w1c", bufs=4)
                        nc.sync.dma_start(w1c[:], w1h[e, f])
                        psA = pf.tile([128, 320], F32, tag="m1", bufs=2)
                        psB = pf.tile([128, 320], F32, tag="m1", bufs=2)
                        for k in range(NK):
                            lw = w1c[:, k * 128:(k + 1) * 128]
                            nc.tensor.matmul(psA[:], lhsT=lw,
                                             rhs=xteb[:, k, 0:320],
                                             start=(k == 0), stop=(k == NK - 1))
                            nc.tensor.matmul(psB[:], lhsT=lw,
                                             rhs=xteb[:, k, 320:640],
                                             start=(k == 0), stop=(k == NK - 1))
                        nc.scalar.activation(ht[f][:, 0:320], psA[:], ACTF.Gelu,
                                             bias=b1_sb[e][:, f:f + 1])
                        nc.scalar.activation(ht[f][:, 320:640], psB[:], ACTF.Gelu,
                                             bias=b1_sb[e][:, f:f + 1])

                    # mm2 + bias + gate-scale; scatter halves at dd 1 and 3
                    yrow = [fp.tile([128, D], F32, tag=f"yrow{ct}", name=f"yrow{ct}",
                                    bufs=1) for ct in range(NCT)]
                    for dd in range(ND):
                        psY = [pf.tile([128, 512], F32, tag=f"m2_{ct}",
                                       name=f"m2_{ct}", bufs=1)
                               for ct in range(NCT)]
                        for f in range(NF):
                            w2c = fp.tile([128, 512], BF16, tag="w2c", bufs=6)
                            nc.sync.dma_start(w2c[:], w2h[e, dd, f])
                            for ct in range(NCT):
                                nc.tensor.matmul(
                                    psY[ct][:],
                                    lhsT=ht[f][:, ct * 128:(ct + 1) * 128],
                                    rhs=w2c[:],
                                    start=(f == 0), stop=(f == NF - 1))
                        for ct in range(NCT):
                            dsl = slice(dd * 512, (dd + 1) * 512)
                            nc.vector.tensor_tensor(out=yrow[ct][:, dsl],
                                                    in0=psY[ct][:],
                                                    in1=b2_sb[e][:, dsl], op=AL.add)
                            nc.vector.tensor_scalar_mul(
                                yrow[ct][:, dsl], yrow[ct][:, dsl],
                                gsb[:, e * NCT + ct:e * NCT + ct + 1])
                            if dd in (1, ND - 1):
                                h0 = 0 if dd == 1 else 1024
                                nc.gpsimd.indirect_dma_start(
                                    out=y[:, :],
                                    out_offset=bass.IndirectOffsetOnAxis(
                                        ap=toki[:, e * NCT + ct:e * NCT + ct + 1],
                                        axis=0),
                                    in_=yrow[ct][:, h0:h0 + 1024], in_offset=None,
                                    element_offset=h0,
                                    bounds_check=T - 1, oob_is_err=False,
                                    compute_op=(AL.bypass if e == 0 else AL.add))

    nc.finalize()
    return nc


def _prep_inputs(x, Wg, W1, b1, W2, b2):
    x = np.asarray(x, np.float32).reshape(T, D)
    xtf = np.asarray(x.T, np.float32)
    xb = np.vstack([x, np.zeros((1, D), np.float32)]).astype(ml_dtypes.bfloat16)
    Wg = np.asarray(Wg, np.float32)
    W1 = np.asarray(W1, np.float32)
    W2 = np.asarray(W2, np.float32)
    b1 = np.asarray(b1, np.float32)
    b2 = np.asarray(b2, np.float32)

    wgp = np.ascontiguousarray(
        Wg.reshape(NK, 128, E).transpose(1, 0, 2).reshape(128, NK * E))
    cb = np.zeros((128, CB_W), np.float32)
    cb[:, CB_TOK:CB_TOK + NB] = (np.arange(NB, dtype=np.float32)[None, :] * 128
                                 + np.arange(128, dtype=np.float32)[:, None])
    cb[:, CB_TRI:CB_TRI + 128] = np.triu(np.ones((128, 128), np.float32))
    cb[:, CB_ID:CB_ID + 128] = np.eye(128, dtype=np.float32)
    cb[:, CB_ONE] = 1.0

    in_maps = []
    for c in range(8):
        el = slice(2 * c, 2 * c + 2)
        w1h = np.ascontiguousarray(
            W1[el].reshape(EL, NK, 128, NF, 128).transpose(0, 3, 2, 1, 4)
        ).astype(ml_dtypes.bfloat16)
        w2h = np.ascontiguousarray(
            W2[el].reshape(EL, NF, 128, ND, 512).transpose(0, 3, 1, 2, 4)
        ).astype(ml_dtypes.bfloat16)
        b1hc = np.ascontiguousarray(b1[el].reshape(EL, NF, 128).transpose(0, 2, 1))
        b2dc = np.ascontiguousarray(b2[el]).astype(ml_dtypes.bfloat16)
        cbc = cb.copy()
        cbc[:, CB_BASE] = float(NSL * c)
        xtloc = np.ascontiguousarray(xtf[:, 512 * c:512 * (c + 1)])
        in_maps.append(dict(xtloc=xtloc, xb=xb, wgp=wgp, w1h=w1h, w2h=w2h,
                            b1h=b1hc, b2d=b2dc, cb=cbc))
    return in_maps


def _run(inputs, trace=False, trace_cores=None):
    if "nc" not in _CACHE:
        _CACHE["nc"] = _build_nc()
    nc = _CACHE["nc"]
    in_maps = _prep_inputs(inputs["x"], inputs["Wg"], inputs["W1"],
                           inputs["b1"], inputs["W2"], inputs["b2"])
    res = run_bass_kernel_spmd(nc, in_maps, list(range(8)), trace=trace,
                               trace_cores=trace_cores)
    y = np.zeros((T, D), np.float64)
    for r in res.results:
        y += r["y"].astype(np.float64)
    y = y.astype(np.float32).reshape(B, S, D)
    return y, res


def kernel(x, Wg, W1, b1, W2, b2):
    y, _ = _run(dict(x=x, Wg=Wg, W1=W1, b1=b1, W2=W2, b2=b2))
    return y
